# revision 11
# baseline (speedup 1.0000x reference)
"""Trainium2 Bass kernel for ComplexDFT256 — radix-2, bf16/fp8-e3m4 mix.

Math: the 256-point complex DFT out = z @ M (z = [xr | xi], M the
512x512 real form of the DFT) is split radix-2 over time samples:
  X[k]      = E[k] + G[k]        k = 0..127
  X[k+128]  = E[k] - G[k]
with E = DFT-128 of even samples and G = (twiddle * DFT-128) of odd
samples; the twiddles fold into G's matrix on the host, so on-device
this is two [B,256]@[256,256] matmuls (half the PE work of the dense
form) plus one add + one sub per output tile (the DVE butterfly).

Precision: the kernel is HBM-bound (measured ~292 GB/s reads / ~328
GB/s writes per NC with all 8 cores streaming), so bytes == time.
Three of the four 128-row contraction blocks stream in fp8 e3m4
(4-bit mantissa; matrix blocks also e3m4), one block stays bf16 —
37.5% less read traffic.  Host-side casts are the only quantization
(fp8 x fp8 products are exact in the fp32 PSUM accumulation), and the
input data is deterministic, so the end-to-end error is a fixed
~1.6e-2 of output norm vs the fp32 reference (tolerance 2e-2;
all-bf16 gives 3.2e-3, all-e3m4 would give ~1.8e-2).  PSUM
accumulates fp32, ACT drains to bf16, outputs store as bf16.

DMA structure: ALL chunk loads are issued up-front on the SP HWDGE
ring; stores are issued on the same ring strictly behind them.  No
store's semaphore wait (gated on compute) ever stalls a load, and the
HBM sees a clean read phase then write phase per iteration (measured
faster than interleaving reads/writes or splitting across rings).
Input is pre-arranged on the host to chunk-major layouts with
per-partition-contiguous descriptors.

Compute per 512-row group: 8 transposed matmuls (stationary = 128x128
M block, moving = 512 batch columns) into two 2-bank PSUM tiles (E
and G halves; pool bufs=4 keeps two groups in flight so each ACT
drain frees banks early — finer PE<->ACT pipelining), two 2-bank
PSUM->SBUF drains on ACT, two all-SBUF bf16 DVE butterfly ops (2x
perf mode), one store.  Compute (~40us) hides fully under DMA.

Sharding: pure data parallel over batch across 8 NeuronCores (8192
rows each).  Host pre-permutes columns to [even | odd] order and lays
out chunk-major; the transposed device output layout is un-permuted
on the host.
"""
import numpy as np
import ml_dtypes

import concourse.bacc as bacc
import concourse.mybir as mybir
import concourse.tile as tile
from concourse.bass_utils import run_bass_kernel_spmd

N_CORES = 8
BATCH = 65536
FFT = 256
C = 2 * FFT            # contraction dim = 512 ([even 256 | odd 256])
J = 2 * FFT            # output features = 512
B_SHARD = BATCH // N_CORES   # 8192
GROUP_B = 512          # batch rows per matmul group (moving free dim)
N_GROUPS = B_SHARD // GROUP_B             # 16
CB = 1024              # batch rows per load chunk
NCH = B_SHARD // CB    # 8
GPC = CB // GROUP_B    # groups per chunk = 2

BF16 = ml_dtypes.bfloat16
F8 = ml_dtypes.float8_e3m4

_cache = {}


def _build_nc(reps: int = 1, unroll: bool = False):
    nc = bacc.Bacc("TRN2", target_bir_lowering=False, debug=False,
                   num_devices=N_CORES)
    f32 = mybir.dt.float32
    bf16 = mybir.dt.bfloat16
    f8 = mybir.dt.float8e3

    # chunk-major inputs: [chunk, partition, (k-block,) batch-in-chunk];
    # contraction block 0 (even-sample first half) in bf16, blocks 1..3
    # in fp8 e3m4
    zt_dram = nc.dram_tensor("zt", [NCH, 128, CB], bf16,
                             kind="ExternalInput")
    z8_dram = nc.dram_tensor("z8", [NCH, 128, 3, CB], f8,
                             kind="ExternalInput")
    m_dram = nc.dram_tensor("m", [128, 256], bf16, kind="ExternalInput")
    m8_dram = nc.dram_tensor("m8", [384, 256], f8, kind="ExternalInput")
    # transposed output: [group, j-partition, lo/hi, Re/Im, batch-in-group];
    # host un-permutes
    out_dram = nc.dram_tensor("out", [N_GROUPS, 128, 2, 2, GROUP_B], bf16,
                              kind="ExternalOutput")

    with tile.TileContext(nc) as tc:
        with (
            tc.tile_pool(name="mpool", bufs=1) as mpool,
            tc.tile_pool(name="zpool", bufs=NCH) as zpool,
            tc.tile_pool(name="gpool", bufs=4) as gpool,
            tc.tile_pool(name="opool", bufs=6) as opool,
            tc.tile_pool(name="psum", bufs=4, space="PSUM") as psum_pool,
        ):
            # SWDGE: keeps the m loads off the SP queue so the first
            # zt chunk streams in parallel
            m0_sb = mpool.tile([128, 256], bf16, tag="m0")
            nc.gpsimd.dma_start(m0_sb[:], m_dram[:])
            m8_sb = []
            for k in range(3):
                mt = mpool.tile([128, 256], f8, tag=f"m8{k}")
                nc.gpsimd.dma_start(mt[:], m8_dram[k * 128:(k + 1) * 128, :])
                m8_sb.append(mt)

            def body():
                # all loads up-front on the SP HWDGE ring; stores queue
                # strictly behind, so no compute-gated store wait ever
                # stalls a load
                zts, z8s = [], []
                for c in range(NCH):
                    zt_sb = zpool.tile([128, CB], bf16, tag="zt")
                    nc.sync.dma_start(zt_sb[:], zt_dram[c])
                    zts.append(zt_sb)
                    z8_sb = zpool.tile([128, 3, CB], f8, tag="z8")
                    nc.sync.dma_start(z8_sb[:], z8_dram[c])
                    z8s.append(z8_sb)
                G = 0
                for c in range(NCH):
                    for g in range(GPC):
                        # Transposed matmuls: stationary = 128x128 M
                        # block, moving = 512 batch columns.  E and G
                        # accumulate into separate 2-bank PSUM tiles
                        # (pool bufs=4 -> two groups in flight) so each
                        # ACT drain frees banks as soon as its half is
                        # done.
                        Y_e = psum_pool.tile([128, 2, GROUP_B], f32,
                                             tag="acc")
                        Y_g = psum_pool.tile([128, 2, GROUP_B], f32,
                                             tag="acc")
                        csl = slice(g * GROUP_B, (g + 1) * GROUP_B)
                        for jt in range(2):
                            jsl = slice(jt * 128, (jt + 1) * 128)
                            nc.tensor.matmul(
                                Y_e[:, jt, :],
                                m0_sb[:, jsl],
                                zts[c][:, csl],
                                start=True, stop=False)
                            nc.tensor.matmul(
                                Y_e[:, jt, :],
                                m8_sb[0][:, jsl],
                                z8s[c][:, 0, csl],
                                start=False, stop=True)
                        for jt in range(2):
                            jsl = slice(jt * 128, (jt + 1) * 128)
                            nc.tensor.matmul(
                                Y_g[:, jt, :],
                                m8_sb[1][:, jsl],
                                z8s[c][:, 1, csl],
                                start=True, stop=False)
                            nc.tensor.matmul(
                                Y_g[:, jt, :],
                                m8_sb[2][:, jsl],
                                z8s[c][:, 2, csl],
                                start=False, stop=True)
                        # two 2-bank PSUM->SBUF drains on ACT (bf16),
                        # then both DVE butterfly ops run all-SBUF in
                        # bf16 (DVE TensorTensor may read only one PSUM
                        # input; all-SBUF 16-bit ops get DVE 2x mode)
                        stg_e = gpool.tile([128, 2, GROUP_B], bf16,
                                           tag="stge")
                        nc.scalar.copy(stg_e[:], Y_e[:])
                        stg_g = gpool.tile([128, 2, GROUP_B], bf16,
                                           tag="stgg")
                        if G % 2 == 0:
                            nc.scalar.copy(stg_g[:], Y_g[:])
                        else:
                            # alternate the G drain onto DVE so ACT's
                            # per-group time stays under the PE pace
                            nc.vector.tensor_copy(stg_g[:], Y_g[:])
                        # out_sb[:, a, jt, :]: a=0 -> X[k]=E+G, a=1 ->
                        # X[k+128]=E-G; contiguous 1024-elem DVE writes
                        out_sb = opool.tile([128, 2, 2, GROUP_B], bf16,
                                            tag="out")
                        nc.vector.tensor_add(out_sb[:, 0, :, :],
                                             stg_e[:], stg_g[:])
                        nc.vector.tensor_sub(out_sb[:, 1, :, :],
                                             stg_e[:], stg_g[:])
                        nc.sync.dma_start(out_dram[G], out_sb[:])
                        G += 1

            if reps == 1:
                body()
            elif unroll:
                for _ in range(reps):
                    body()
            else:
                with tc.For_i(0, reps, 1):
                    body()

    nc.compile()
    return nc


def _get_nc():
    if "nc" not in _cache:
        _cache["nc"] = _build_nc()
    return _cache["nc"]


def _prepare_in_maps(x, cos_kernel, sin_kernel):
    x = np.asarray(x, dtype=np.float32)
    cos = np.asarray(cos_kernel, dtype=np.float32)
    sin = np.asarray(sin_kernel, dtype=np.float32)

    m = np.empty((C, J), dtype=np.float32)
    m[:FFT, :FFT] = cos.T
    m[:FFT, FFT:] = sin.T
    m[FFT:, :FFT] = -sin.T
    m[FFT:, FFT:] = cos.T

    # radix-2: even/odd sample rows; cols k<128 of both Re and Im halves
    # (cols k+128 equal these up to the sign of the odd-row block)
    rows_e = np.concatenate([np.arange(0, 256, 2), np.arange(256, 512, 2)])
    rows_o = rows_e + 1
    cols_lo = np.concatenate([np.arange(0, 128), np.arange(256, 384)])
    me = m[np.ix_(rows_e, cols_lo)]     # [256, 256]
    mg = m[np.ix_(rows_o, cols_lo)]     # [256, 256]
    m_full = np.concatenate([me, mg], axis=0)              # [512, 256] f32
    m_dev = np.ascontiguousarray(m_full[:128]).astype(BF16)
    m8_dev = np.ascontiguousarray(m_full[128:]).astype(F8)

    z = x.reshape(BATCH, C)[:, np.concatenate([rows_e, rows_o])]  # f32
    # chunk-major per-core layouts:
    # zt[cc, p, j]    = z[core*B_SHARD + cc*CB + j, p]            (bf16)
    # z8[cc, p, k, j] = z[core*B_SHARD + cc*CB + j, 128(k+1) + p] (fp8)
    z16 = z[:, :128].astype(BF16)
    zc16 = (z16.view(np.uint16)
            .reshape(N_CORES, NCH, CB, 128)
            .transpose(0, 1, 3, 2))
    zc16 = np.ascontiguousarray(zc16)   # [N_CORES, NCH, 128, CB] u16
    z8 = z[:, 128:].astype(F8)
    zc8 = (z8.view(np.uint8)
           .reshape(N_CORES, NCH, CB, 3, 128)
           .transpose(0, 1, 4, 3, 2))
    zc8 = np.ascontiguousarray(zc8)     # [N_CORES, NCH, 128, 3, CB] u8

    in_maps = []
    for c in range(N_CORES):
        in_maps.append({"zt": zc16[c].view(BF16), "z8": zc8[c].view(F8),
                        "m": m_dev, "m8": m8_dev})
    return in_maps


def _run(in_maps, trace=False):
    nc = _get_nc()
    return run_bass_kernel_spmd(nc, in_maps, list(range(N_CORES)), trace=trace)


def kernel(x, cos_kernel, sin_kernel):
    in_maps = _prepare_in_maps(x, cos_kernel, sin_kernel)
    res = _run(in_maps)
    outs = []
    for r in res.results:
        # [G, p, a, q, b] bf16 (transposed): row = G*GROUP_B + b,
        # col = q*256 + a*128 + p
        o = np.asarray(r["out"]).view(np.uint16)
        o = o.transpose(0, 4, 3, 2, 1).reshape(B_SHARD, J)  # (G,b,q,a,p)
        outs.append(o)
    out = np.concatenate(outs, axis=0).view(BF16).astype(np.float32)
    return out.reshape(BATCH, J, 1)


# revision 12
# speedup vs baseline: 1.0113x; 1.0113x over previous
"""Trainium2 Bass kernel for ComplexDFT256 — radix-2, bf16/fp8-e3m4 mix.

Math: the 256-point complex DFT out = z @ M (z = [xr | xi], M the
512x512 real form of the DFT) is split radix-2 over time samples:
  X[k]      = E[k] + G[k]        k = 0..127
  X[k+128]  = E[k] - G[k]
with E = DFT-128 of even samples and G = (twiddle * DFT-128) of odd
samples; the twiddles fold into G's matrix on the host, so on-device
this is two [B,256]@[256,256] matmuls (half the PE work of the dense
form) plus one add + one sub per output tile (the DVE butterfly).

Precision: the kernel is HBM-bound (measured ~292 GB/s reads / ~328
GB/s writes per NC with all 8 cores streaming), so bytes == time.
Three of the four 128-row contraction blocks stream in fp8 e3m4
(4-bit mantissa; matrix blocks also e3m4), one block stays bf16 —
37.5% less read traffic.  Host-side casts are the only quantization
(fp8 x fp8 products are exact in the fp32 PSUM accumulation), and the
input data is deterministic, so the end-to-end error is a fixed
~1.6e-2 of output norm vs the fp32 reference (tolerance 2e-2;
all-bf16 gives 3.2e-3, all-e3m4 would give ~1.8e-2).  PSUM
accumulates fp32, ACT drains to bf16, outputs store as bf16.

DMA structure: ALL chunk loads are issued up-front on the SP HWDGE
ring; stores are issued on the same ring strictly behind them.  No
store's semaphore wait (gated on compute) ever stalls a load, and the
HBM sees a clean read phase then write phase per iteration (measured
faster than interleaving reads/writes or splitting across rings).
Input is pre-arranged on the host to chunk-major layouts with
per-partition-contiguous descriptors.

Compute per 512-row group: 8 transposed matmuls (stationary = 128x128
M block, moving = 512 batch columns) into two 2-bank PSUM tiles (E
and G halves; pool bufs=4 keeps two groups in flight so each ACT
drain frees banks early — finer PE<->ACT pipelining), two 2-bank
PSUM->SBUF drains on ACT, two all-SBUF bf16 DVE butterfly ops (2x
perf mode), one store.  Compute (~40us) hides fully under DMA.

Sharding: pure data parallel over batch across 8 NeuronCores (8192
rows each).  Host pre-permutes columns to [even | odd] order and lays
out chunk-major; the transposed device output layout is un-permuted
on the host.
"""
import numpy as np
import ml_dtypes

import concourse.bacc as bacc
import concourse.mybir as mybir
import concourse.tile as tile
from concourse.bass_utils import run_bass_kernel_spmd

N_CORES = 8
BATCH = 65536
FFT = 256
C = 2 * FFT            # contraction dim = 512 ([even 256 | odd 256])
J = 2 * FFT            # output features = 512
B_SHARD = BATCH // N_CORES   # 8192
GROUP_B = 512          # batch rows per matmul group (moving free dim)
N_GROUPS = B_SHARD // GROUP_B             # 16
CB = 1024              # batch rows per load chunk
NCH = B_SHARD // CB    # 8
GPC = CB // GROUP_B    # groups per chunk = 2

BF16 = ml_dtypes.bfloat16
F8 = ml_dtypes.float8_e3m4

_cache = {}


def _build_nc(reps: int = 1, unroll: bool = False):
    nc = bacc.Bacc("TRN2", target_bir_lowering=False, debug=False,
                   num_devices=N_CORES)
    f32 = mybir.dt.float32
    bf16 = mybir.dt.bfloat16
    f8 = mybir.dt.float8e3

    # chunk-major inputs: [chunk, partition, (k-block,) batch-in-chunk];
    # contraction block 0 (even-sample first half) in bf16, blocks 1..3
    # in fp8 e3m4
    zt_dram = nc.dram_tensor("zt", [NCH, 128, CB], bf16,
                             kind="ExternalInput")
    z8_dram = nc.dram_tensor("z8", [NCH, 128, 3, CB], f8,
                             kind="ExternalInput")
    m_dram = nc.dram_tensor("m", [128, 256], bf16, kind="ExternalInput")
    m8_dram = nc.dram_tensor("m8", [384, 256], f8, kind="ExternalInput")
    # transposed output: [group, j-partition, lo/hi, Re/Im, batch-in-group];
    # host un-permutes
    out_dram = nc.dram_tensor("out", [N_GROUPS, 128, 2, 2, GROUP_B], bf16,
                              kind="ExternalOutput")

    with tile.TileContext(nc) as tc:
        with (
            tc.tile_pool(name="mpool", bufs=1) as mpool,
            tc.tile_pool(name="zpool", bufs=NCH) as zpool,
            tc.tile_pool(name="gpool", bufs=4) as gpool,
            tc.tile_pool(name="opool", bufs=6) as opool,
            tc.tile_pool(name="psum", bufs=4, space="PSUM") as psum_pool,
        ):
            # SWDGE: keeps the m loads off the SP queue so the first
            # zt chunk streams in parallel
            m0_sb = mpool.tile([128, 256], bf16, tag="m0")
            nc.gpsimd.dma_start(m0_sb[:], m_dram[:])
            m8_sb = []
            for k in range(3):
                mt = mpool.tile([128, 256], f8, tag=f"m8{k}")
                nc.gpsimd.dma_start(mt[:], m8_dram[k * 128:(k + 1) * 128, :])
                m8_sb.append(mt)

            def body():
                # all loads up-front on the SP HWDGE ring; stores queue
                # strictly behind, so no compute-gated store wait ever
                # stalls a load
                zts, z8s = [], []
                for c in range(NCH):
                    zt_sb = zpool.tile([128, CB], bf16, tag="zt")
                    nc.sync.dma_start(zt_sb[:], zt_dram[c])
                    zts.append(zt_sb)
                    z8_sb = zpool.tile([128, 3, CB], f8, tag="z8")
                    nc.sync.dma_start(z8_sb[:], z8_dram[c])
                    z8s.append(z8_sb)
                G = 0
                for c in range(NCH):
                    for g in range(GPC):
                        # Transposed matmuls: stationary = 128x128 M
                        # block, moving = 512 batch columns.  E and G
                        # accumulate into separate 2-bank PSUM tiles
                        # (pool bufs=4 -> two groups in flight) so each
                        # ACT drain frees banks as soon as its half is
                        # done.
                        Y_e = psum_pool.tile([128, 2, GROUP_B], f32,
                                             tag="acc")
                        Y_g = psum_pool.tile([128, 2, GROUP_B], f32,
                                             tag="acc")
                        csl = slice(g * GROUP_B, (g + 1) * GROUP_B)
                        for jt in range(2):
                            jsl = slice(jt * 128, (jt + 1) * 128)
                            nc.tensor.matmul(
                                Y_e[:, jt, :],
                                m0_sb[:, jsl],
                                zts[c][:, csl],
                                start=True, stop=False)
                            nc.tensor.matmul(
                                Y_e[:, jt, :],
                                m8_sb[0][:, jsl],
                                z8s[c][:, 0, csl],
                                start=False, stop=True)
                        for jt in range(2):
                            jsl = slice(jt * 128, (jt + 1) * 128)
                            nc.tensor.matmul(
                                Y_g[:, jt, :],
                                m8_sb[1][:, jsl],
                                z8s[c][:, 1, csl],
                                start=True, stop=False)
                            nc.tensor.matmul(
                                Y_g[:, jt, :],
                                m8_sb[2][:, jsl],
                                z8s[c][:, 2, csl],
                                start=False, stop=True)
                        # two 2-bank PSUM->SBUF drains on ACT (bf16),
                        # then both DVE butterfly ops run all-SBUF in
                        # bf16 (DVE TensorTensor may read only one PSUM
                        # input; all-SBUF 16-bit ops get DVE 2x mode)
                        stg_e = gpool.tile([128, 2, GROUP_B], bf16,
                                           tag="stge")
                        nc.scalar.copy(stg_e[:], Y_e[:])
                        stg_g = gpool.tile([128, 2, GROUP_B], bf16,
                                           tag="stgg")
                        nc.scalar.copy(stg_g[:], Y_g[:])
                        # out_sb[:, a, jt, :]: a=0 -> X[k]=E+G, a=1 ->
                        # X[k+128]=E-G; contiguous 1024-elem DVE writes
                        out_sb = opool.tile([128, 2, 2, GROUP_B], bf16,
                                            tag="out")
                        nc.vector.tensor_add(out_sb[:, 0, :, :],
                                             stg_e[:], stg_g[:])
                        nc.vector.tensor_sub(out_sb[:, 1, :, :],
                                             stg_e[:], stg_g[:])
                        nc.sync.dma_start(out_dram[G], out_sb[:])
                        G += 1

            if reps == 1:
                body()
            elif unroll:
                for _ in range(reps):
                    body()
            else:
                with tc.For_i(0, reps, 1):
                    body()

    nc.compile()
    return nc


def _get_nc():
    if "nc" not in _cache:
        _cache["nc"] = _build_nc()
    return _cache["nc"]


def _prepare_in_maps(x, cos_kernel, sin_kernel):
    x = np.asarray(x, dtype=np.float32)
    cos = np.asarray(cos_kernel, dtype=np.float32)
    sin = np.asarray(sin_kernel, dtype=np.float32)

    m = np.empty((C, J), dtype=np.float32)
    m[:FFT, :FFT] = cos.T
    m[:FFT, FFT:] = sin.T
    m[FFT:, :FFT] = -sin.T
    m[FFT:, FFT:] = cos.T

    # radix-2: even/odd sample rows; cols k<128 of both Re and Im halves
    # (cols k+128 equal these up to the sign of the odd-row block)
    rows_e = np.concatenate([np.arange(0, 256, 2), np.arange(256, 512, 2)])
    rows_o = rows_e + 1
    cols_lo = np.concatenate([np.arange(0, 128), np.arange(256, 384)])
    me = m[np.ix_(rows_e, cols_lo)]     # [256, 256]
    mg = m[np.ix_(rows_o, cols_lo)]     # [256, 256]
    m_full = np.concatenate([me, mg], axis=0)              # [512, 256] f32
    m_dev = np.ascontiguousarray(m_full[:128]).astype(BF16)
    m8_dev = np.ascontiguousarray(m_full[128:]).astype(F8)

    z = x.reshape(BATCH, C)[:, np.concatenate([rows_e, rows_o])]  # f32
    # chunk-major per-core layouts:
    # zt[cc, p, j]    = z[core*B_SHARD + cc*CB + j, p]            (bf16)
    # z8[cc, p, k, j] = z[core*B_SHARD + cc*CB + j, 128(k+1) + p] (fp8)
    z16 = z[:, :128].astype(BF16)
    zc16 = (z16.view(np.uint16)
            .reshape(N_CORES, NCH, CB, 128)
            .transpose(0, 1, 3, 2))
    zc16 = np.ascontiguousarray(zc16)   # [N_CORES, NCH, 128, CB] u16
    z8 = z[:, 128:].astype(F8)
    zc8 = (z8.view(np.uint8)
           .reshape(N_CORES, NCH, CB, 3, 128)
           .transpose(0, 1, 4, 3, 2))
    zc8 = np.ascontiguousarray(zc8)     # [N_CORES, NCH, 128, 3, CB] u8

    in_maps = []
    for c in range(N_CORES):
        in_maps.append({"zt": zc16[c].view(BF16), "z8": zc8[c].view(F8),
                        "m": m_dev, "m8": m8_dev})
    return in_maps


def _run(in_maps, trace=False):
    nc = _get_nc()
    return run_bass_kernel_spmd(nc, in_maps, list(range(N_CORES)), trace=trace)


def kernel(x, cos_kernel, sin_kernel):
    in_maps = _prepare_in_maps(x, cos_kernel, sin_kernel)
    res = _run(in_maps)
    outs = []
    for r in res.results:
        # [G, p, a, q, b] bf16 (transposed): row = G*GROUP_B + b,
        # col = q*256 + a*128 + p
        o = np.asarray(r["out"]).view(np.uint16)
        o = o.transpose(0, 4, 3, 2, 1).reshape(B_SHARD, J)  # (G,b,q,a,p)
        outs.append(o)
    out = np.concatenate(outs, axis=0).view(BF16).astype(np.float32)
    return out.reshape(BATCH, J, 1)


# revision 14
# speedup vs baseline: 1.0498x; 1.0381x over previous
"""Trainium2 Bass kernel for ComplexDFT256 — radix-2, bf16/fp8-e3m4 mix.

Math: the 256-point complex DFT out = z @ M (z = [xr | xi], M the
512x512 real form of the DFT) is split radix-2 over time samples:
  X[k]      = E[k] + G[k]        k = 0..127
  X[k+128]  = E[k] - G[k]
with E = DFT-128 of even samples and G = (twiddle * DFT-128) of odd
samples; the twiddles fold into G's matrix on the host, so on-device
this is two [B,256]@[256,256] matmuls (half the PE work of the dense
form) plus one add + one sub per output tile (the DVE butterfly).

Precision: the kernel is HBM-bound (measured ~292 GB/s reads / ~328
GB/s writes per NC with all 8 cores streaming), so bytes == time.
Three of the four 128-row contraction blocks stream in fp8 e3m4
(4-bit mantissa; matrix blocks also e3m4), one block stays bf16 —
37.5% less read traffic.  Host-side casts are the only quantization
(fp8 x fp8 products are exact in the fp32 PSUM accumulation), and the
input data is deterministic, so the end-to-end error is a fixed
~1.6e-2 of output norm vs the fp32 reference (tolerance 2e-2;
all-bf16 gives 3.2e-3, all-e3m4 would give ~1.8e-2).  PSUM
accumulates fp32, ACT drains to bf16, outputs store as bf16.

DMA structure: ALL chunk loads are issued up-front on the SP HWDGE
ring; stores are issued on the same ring strictly behind them.  No
store's semaphore wait (gated on compute) ever stalls a load, and the
HBM sees a clean read phase then write phase per iteration (measured
faster than interleaving reads/writes or splitting across rings).
Input is pre-arranged on the host to chunk-major layouts with
per-partition-contiguous descriptors.

Compute per 512-row group: 8 transposed matmuls (stationary = 128x128
M block, moving = 512 batch columns) into two 2-bank PSUM tiles (E
and G halves; pool bufs=4 keeps two groups in flight so each ACT
drain frees banks early — finer PE<->ACT pipelining), two 2-bank
PSUM->SBUF drains on ACT, two all-SBUF bf16 DVE butterfly ops (2x
perf mode), one store.  Compute (~40us) hides fully under DMA.

Sharding: pure data parallel over batch across 8 NeuronCores (8192
rows each).  Host pre-permutes columns to [even | odd] order and lays
out chunk-major; the transposed device output layout is un-permuted
on the host.
"""
import numpy as np
import ml_dtypes

import concourse.bacc as bacc
import concourse.mybir as mybir
import concourse.tile as tile
from concourse.bass_utils import run_bass_kernel_spmd

N_CORES = 8
BATCH = 65536
FFT = 256
C = 2 * FFT            # contraction dim = 512 ([even 256 | odd 256])
J = 2 * FFT            # output features = 512
B_SHARD = BATCH // N_CORES   # 8192
GROUP_B = 512          # batch rows per matmul group (moving free dim)
N_GROUPS = B_SHARD // GROUP_B             # 16
CB = 1024              # batch rows per load chunk
NCH = B_SHARD // CB    # 8
GPC = CB // GROUP_B    # groups per chunk = 2

BF16 = ml_dtypes.bfloat16
F8 = ml_dtypes.float8_e3m4

_cache = {}


def _build_nc(reps: int = 1, unroll: bool = False):
    nc = bacc.Bacc("TRN2", target_bir_lowering=False, debug=False,
                   num_devices=N_CORES)
    f32 = mybir.dt.float32
    bf16 = mybir.dt.bfloat16
    f8 = mybir.dt.float8e3

    # chunk-major inputs: [chunk, partition, (k-block,) batch-in-chunk];
    # contraction block 0 (even-sample first half) in bf16, blocks 1..3
    # in fp8 e3m4
    zt_dram = nc.dram_tensor("zt", [NCH, 128, CB], bf16,
                             kind="ExternalInput")
    z8_dram = nc.dram_tensor("z8", [NCH, 128, 3, CB], f8,
                             kind="ExternalInput")
    m_dram = nc.dram_tensor("m", [128, 256], bf16, kind="ExternalInput")
    m8_dram = nc.dram_tensor("m8", [384, 256], f8, kind="ExternalInput")
    # transposed output: [group, j-partition, lo/hi, Re/Im, batch-in-group];
    # host un-permutes
    out_dram = nc.dram_tensor("out", [N_GROUPS, 128, 2, 2, GROUP_B], bf16,
                              kind="ExternalOutput")

    with tile.TileContext(nc) as tc:
        with (
            tc.tile_pool(name="mpool", bufs=1) as mpool,
            tc.tile_pool(name="zpool", bufs=NCH) as zpool,
            tc.tile_pool(name="gpool", bufs=4) as gpool,
            tc.tile_pool(name="opool", bufs=6) as opool,
            tc.tile_pool(name="psum", bufs=4, space="PSUM") as psum_pool,
        ):
            # SWDGE: keeps the m loads off the SP queue so the first
            # zt chunk streams in parallel
            m0_sb = mpool.tile([128, 256], bf16, tag="m0")
            nc.gpsimd.dma_start(m0_sb[:], m_dram[:])
            m8_sb = []
            for k in range(3):
                mt = mpool.tile([128, 256], f8, tag=f"m8{k}")
                nc.gpsimd.dma_start(mt[:], m8_dram[k * 128:(k + 1) * 128, :])
                m8_sb.append(mt)

            def body():
                # all loads up-front on the SP HWDGE ring; stores queue
                # strictly behind, so no compute-gated store wait ever
                # stalls a load
                zts, z8s = [], []
                for c in range(NCH):
                    zt_sb = zpool.tile([128, CB], bf16, tag="zt")
                    nc.sync.dma_start(zt_sb[:], zt_dram[c])
                    zts.append(zt_sb)
                    z8_sb = zpool.tile([128, 3, CB], f8, tag="z8")
                    nc.sync.dma_start(z8_sb[:], z8_dram[c])
                    z8s.append(z8_sb)
                G = 0
                for c in range(NCH):
                    for g in range(GPC):
                        # Transposed matmuls: stationary = 128x128 M
                        # block, moving = 512 batch columns.  E and G
                        # accumulate into separate 2-bank PSUM tiles
                        # (pool bufs=4 -> two groups in flight) so each
                        # ACT drain frees banks as soon as its half is
                        # done.
                        Y_e = psum_pool.tile([128, 2, GROUP_B], f32,
                                             tag="acc")
                        Y_g = psum_pool.tile([128, 2, GROUP_B], f32,
                                             tag="acc")
                        csl = slice(g * GROUP_B, (g + 1) * GROUP_B)
                        for jt in range(2):
                            jsl = slice(jt * 128, (jt + 1) * 128)
                            nc.tensor.matmul(
                                Y_e[:, jt, :],
                                m0_sb[:, jsl],
                                zts[c][:, csl],
                                start=True, stop=False)
                            nc.tensor.matmul(
                                Y_e[:, jt, :],
                                m8_sb[0][:, jsl],
                                z8s[c][:, 0, csl],
                                start=False, stop=True)
                        for jt in range(2):
                            jsl = slice(jt * 128, (jt + 1) * 128)
                            nc.tensor.matmul(
                                Y_g[:, jt, :],
                                m8_sb[1][:, jsl],
                                z8s[c][:, 1, csl],
                                start=True, stop=False)
                            nc.tensor.matmul(
                                Y_g[:, jt, :],
                                m8_sb[2][:, jsl],
                                z8s[c][:, 2, csl],
                                start=False, stop=True)
                        # two 2-bank PSUM->SBUF drains on ACT (bf16),
                        # then both DVE butterfly ops run all-SBUF in
                        # bf16 (DVE TensorTensor may read only one PSUM
                        # input; all-SBUF 16-bit ops get DVE 2x mode)
                        stg_e = gpool.tile([128, 2, GROUP_B], bf16,
                                           tag="stge")
                        nc.scalar.copy(stg_e[:], Y_e[:])
                        stg_g = gpool.tile([128, 2, GROUP_B], bf16,
                                           tag="stgg")
                        nc.scalar.copy(stg_g[:], Y_g[:])
                        # out_sb[:, a, jt, :]: a=0 -> X[k]=E+G, a=1 ->
                        # X[k+128]=E-G; contiguous 1024-elem DVE writes
                        out_sb = opool.tile([128, 2, 2, GROUP_B], bf16,
                                            tag="out")
                        nc.vector.tensor_add(out_sb[:, 0, :, :],
                                             stg_e[:], stg_g[:])
                        nc.vector.tensor_sub(out_sb[:, 1, :, :],
                                             stg_e[:], stg_g[:])
                        nc.sync.dma_start(out_dram[G], out_sb[:])
                        G += 1

            if reps == 1:
                body()
            elif unroll:
                for _ in range(reps):
                    body()
            else:
                with tc.For_i(0, reps, 1):
                    body()

    nc.compile()
    return nc


def _get_nc():
    if "nc" not in _cache:
        _cache["nc"] = _build_nc()
    return _cache["nc"]


def _prepare_in_maps(x, cos_kernel, sin_kernel):
    x = np.asarray(x, dtype=np.float32)
    cos = np.asarray(cos_kernel, dtype=np.float32)
    sin = np.asarray(sin_kernel, dtype=np.float32)

    m = np.empty((C, J), dtype=np.float32)
    m[:FFT, :FFT] = cos.T
    m[:FFT, FFT:] = sin.T
    m[FFT:, :FFT] = -sin.T
    m[FFT:, FFT:] = cos.T

    # radix-2: even/odd sample rows; cols k<128 of both Re and Im halves
    # (cols k+128 equal these up to the sign of the odd-row block)
    rows_e = np.concatenate([np.arange(0, 256, 2), np.arange(256, 512, 2)])
    rows_o = rows_e + 1
    cols_lo = np.concatenate([np.arange(0, 128), np.arange(256, 384)])
    me = m[np.ix_(rows_e, cols_lo)]     # [256, 256]
    mg = m[np.ix_(rows_o, cols_lo)]     # [256, 256]
    m_full = np.concatenate([me, mg], axis=0)              # [512, 256] f32
    m_dev = np.ascontiguousarray(m_full[:128]).astype(BF16)
    m8_dev = np.ascontiguousarray(m_full[128:]).astype(F8)

    z = x.reshape(BATCH, C)[:, np.concatenate([rows_e, rows_o])]  # f32
    # chunk-major per-core layouts:
    # zt[cc, p, j]    = z[core*B_SHARD + cc*CB + j, p]            (bf16)
    # z8[cc, p, k, j] = z[core*B_SHARD + cc*CB + j, 128(k+1) + p] (fp8)
    z16 = z[:, :128].astype(BF16)
    zc16 = (z16.view(np.uint16)
            .reshape(N_CORES, NCH, CB, 128)
            .transpose(0, 1, 3, 2))
    zc16 = np.ascontiguousarray(zc16)   # [N_CORES, NCH, 128, CB] u16
    z8 = z[:, 128:].astype(F8)
    zc8 = (z8.view(np.uint8)
           .reshape(N_CORES, NCH, CB, 3, 128)
           .transpose(0, 1, 4, 3, 2))
    zc8 = np.ascontiguousarray(zc8)     # [N_CORES, NCH, 128, 3, CB] u8

    in_maps = []
    for c in range(N_CORES):
        in_maps.append({"zt": zc16[c].view(BF16), "z8": zc8[c].view(F8),
                        "m": m_dev, "m8": m8_dev})
    return in_maps


def _run(in_maps, trace=False):
    nc = _get_nc()
    return run_bass_kernel_spmd(nc, in_maps, list(range(N_CORES)), trace=trace)


def kernel(x, cos_kernel, sin_kernel):
    in_maps = _prepare_in_maps(x, cos_kernel, sin_kernel)
    res = _run(in_maps)
    outs = []
    for r in res.results:
        # [G, p, a, q, b] bf16 (transposed): row = G*GROUP_B + b,
        # col = q*256 + a*128 + p
        o = np.asarray(r["out"]).view(np.uint16)
        o = o.transpose(0, 4, 3, 2, 1).reshape(B_SHARD, J)  # (G,b,q,a,p)
        outs.append(o)
    out = np.concatenate(outs, axis=0).view(BF16).astype(np.float32)
    return out.reshape(BATCH, J, 1)


# revision 15
# speedup vs baseline: 1.0680x; 1.0173x over previous
"""Trainium2 Bass kernel for ComplexDFT256 — radix-2, bf16/fp8-e3m4 mix.

Math: the 256-point complex DFT out = z @ M (z = [xr | xi], M the
512x512 real form of the DFT) is split radix-2 over time samples:
  X[k]      = E[k] + G[k]        k = 0..127
  X[k+128]  = E[k] - G[k]
with E = DFT-128 of even samples and G = (twiddle * DFT-128) of odd
samples; the twiddles fold into G's matrix on the host, so on-device
this is two [B,256]@[256,256] matmuls (half the PE work of the dense
form) plus one add + one sub per output tile (the DVE butterfly).

Precision: the kernel is HBM-bound (measured ~292 GB/s reads / ~328
GB/s writes per NC with all 8 cores streaming), so bytes == time.
Three of the four 128-row contraction blocks stream in fp8 e3m4
(4-bit mantissa; matrix blocks also e3m4), one block stays bf16 —
37.5% less read traffic.  Host-side casts are the only quantization
(fp8 x fp8 products are exact in the fp32 PSUM accumulation), and the
input data is deterministic, so the end-to-end error is a fixed
~1.6e-2 of output norm vs the fp32 reference (tolerance 2e-2;
all-bf16 gives 3.2e-3, all-e3m4 would give ~1.8e-2).  PSUM
accumulates fp32, ACT drains to bf16, outputs store as bf16.

DMA structure: ALL chunk loads are issued up-front on the SP HWDGE
ring; stores are issued on the same ring strictly behind them.  No
store's semaphore wait (gated on compute) ever stalls a load, and the
HBM sees a clean read phase then write phase per iteration (measured
faster than interleaving reads/writes or splitting across rings).
Input is pre-arranged on the host to chunk-major layouts with
per-partition-contiguous descriptors.

Compute per 512-row group: 8 transposed matmuls (stationary = 128x128
M block, moving = 512 batch columns) into two 2-bank PSUM tiles (E
and G halves; pool bufs=4 keeps two groups in flight so each ACT
drain frees banks early — finer PE<->ACT pipelining), two 2-bank
PSUM->SBUF drains on ACT, two all-SBUF bf16 DVE butterfly ops (2x
perf mode), one store.  Compute (~40us) hides fully under DMA.

Sharding: pure data parallel over batch across 8 NeuronCores (8192
rows each).  Host pre-permutes columns to [even | odd] order and lays
out chunk-major; the transposed device output layout is un-permuted
on the host.
"""
import numpy as np
import ml_dtypes

import concourse.bacc as bacc
import concourse.mybir as mybir
import concourse.tile as tile
from concourse.bass_utils import run_bass_kernel_spmd

N_CORES = 8
BATCH = 65536
FFT = 256
C = 2 * FFT            # contraction dim = 512 ([even 256 | odd 256])
J = 2 * FFT            # output features = 512
B_SHARD = BATCH // N_CORES   # 8192
GROUP_B = 512          # batch rows per matmul group (moving free dim)
N_GROUPS = B_SHARD // GROUP_B             # 16
CB = 1024              # batch rows per load chunk
NCH = B_SHARD // CB    # 8
GPC = CB // GROUP_B    # groups per chunk = 2

BF16 = ml_dtypes.bfloat16
F8 = ml_dtypes.float8_e3m4

_cache = {}


def _build_nc(reps: int = 1, unroll: bool = False):
    nc = bacc.Bacc("TRN2", target_bir_lowering=False, debug=False,
                   num_devices=N_CORES)
    f32 = mybir.dt.float32
    bf16 = mybir.dt.bfloat16
    f8 = mybir.dt.float8e3

    # chunk-major inputs: [chunk, partition, (k-block,) batch-in-chunk];
    # contraction block 0 (even-sample first half) in bf16, blocks 1..3
    # in fp8 e3m4
    zt_dram = nc.dram_tensor("zt", [NCH, 128, CB], bf16,
                             kind="ExternalInput")
    z8_dram = nc.dram_tensor("z8", [NCH, 128, 3, CB], f8,
                             kind="ExternalInput")
    m_dram = nc.dram_tensor("m", [128, 256], bf16, kind="ExternalInput")
    m8_dram = nc.dram_tensor("m8", [384, 256], f8, kind="ExternalInput")
    # transposed output: [group, j-partition, lo/hi, Re/Im, batch-in-group];
    # host un-permutes
    out_dram = nc.dram_tensor("out", [N_GROUPS, 128, 2, 2, GROUP_B], bf16,
                              kind="ExternalOutput")

    with tile.TileContext(nc) as tc:
        with (
            tc.tile_pool(name="mpool", bufs=1) as mpool,
            tc.tile_pool(name="zpool", bufs=NCH) as zpool,
            tc.tile_pool(name="gpool", bufs=8) as gpool,
            tc.tile_pool(name="opool", bufs=12) as opool,
            tc.tile_pool(name="psum", bufs=4, space="PSUM") as psum_pool,
        ):
            # SWDGE: keeps the m loads off the SP queue so the first
            # zt chunk streams in parallel
            m0_sb = mpool.tile([128, 256], bf16, tag="m0")
            nc.gpsimd.dma_start(m0_sb[:], m_dram[:])
            m8_sb = []
            for k in range(3):
                mt = mpool.tile([128, 256], f8, tag=f"m8{k}")
                nc.gpsimd.dma_start(mt[:], m8_dram[k * 128:(k + 1) * 128, :])
                m8_sb.append(mt)

            def body():
                # all loads up-front on the SP HWDGE ring; stores queue
                # strictly behind, so no compute-gated store wait ever
                # stalls a load
                zts, z8s = [], []
                for c in range(NCH):
                    zt_sb = zpool.tile([128, CB], bf16, tag="zt")
                    nc.sync.dma_start(zt_sb[:], zt_dram[c])
                    zts.append(zt_sb)
                    z8_sb = zpool.tile([128, 3, CB], f8, tag="z8")
                    nc.sync.dma_start(z8_sb[:], z8_dram[c])
                    z8s.append(z8_sb)
                G = 0
                for c in range(NCH):
                    for g in range(GPC):
                        # Transposed matmuls: stationary = 128x128 M
                        # block, moving = 512 batch columns.  E and G
                        # accumulate into separate 2-bank PSUM tiles
                        # (pool bufs=4 -> two groups in flight) so each
                        # ACT drain frees banks as soon as its half is
                        # done.
                        Y_e = psum_pool.tile([128, 2, GROUP_B], f32,
                                             tag="acc")
                        Y_g = psum_pool.tile([128, 2, GROUP_B], f32,
                                             tag="acc")
                        csl = slice(g * GROUP_B, (g + 1) * GROUP_B)
                        for jt in range(2):
                            jsl = slice(jt * 128, (jt + 1) * 128)
                            nc.tensor.matmul(
                                Y_e[:, jt, :],
                                m0_sb[:, jsl],
                                zts[c][:, csl],
                                start=True, stop=False)
                            nc.tensor.matmul(
                                Y_e[:, jt, :],
                                m8_sb[0][:, jsl],
                                z8s[c][:, 0, csl],
                                start=False, stop=True)
                        for jt in range(2):
                            jsl = slice(jt * 128, (jt + 1) * 128)
                            nc.tensor.matmul(
                                Y_g[:, jt, :],
                                m8_sb[1][:, jsl],
                                z8s[c][:, 1, csl],
                                start=True, stop=False)
                            nc.tensor.matmul(
                                Y_g[:, jt, :],
                                m8_sb[2][:, jsl],
                                z8s[c][:, 2, csl],
                                start=False, stop=True)
                        # two 2-bank PSUM->SBUF drains on ACT (bf16),
                        # then both DVE butterfly ops run all-SBUF in
                        # bf16 (DVE TensorTensor may read only one PSUM
                        # input; all-SBUF 16-bit ops get DVE 2x mode)
                        stg_e = gpool.tile([128, 2, GROUP_B], bf16,
                                           tag="stge")
                        nc.scalar.copy(stg_e[:], Y_e[:])
                        stg_g = gpool.tile([128, 2, GROUP_B], bf16,
                                           tag="stgg")
                        nc.scalar.copy(stg_g[:], Y_g[:])
                        # out_sb[:, a, jt, :]: a=0 -> X[k]=E+G, a=1 ->
                        # X[k+128]=E-G; contiguous 1024-elem DVE writes
                        out_sb = opool.tile([128, 2, 2, GROUP_B], bf16,
                                            tag="out")
                        nc.vector.tensor_add(out_sb[:, 0, :, :],
                                             stg_e[:], stg_g[:])
                        nc.vector.tensor_sub(out_sb[:, 1, :, :],
                                             stg_e[:], stg_g[:])
                        nc.sync.dma_start(out_dram[G], out_sb[:])
                        G += 1

            if reps == 1:
                body()
            elif unroll:
                for _ in range(reps):
                    body()
            else:
                with tc.For_i(0, reps, 1):
                    body()

    nc.compile()
    return nc


def _get_nc():
    if "nc" not in _cache:
        _cache["nc"] = _build_nc()
    return _cache["nc"]


def _prepare_in_maps(x, cos_kernel, sin_kernel):
    x = np.asarray(x, dtype=np.float32)
    cos = np.asarray(cos_kernel, dtype=np.float32)
    sin = np.asarray(sin_kernel, dtype=np.float32)

    m = np.empty((C, J), dtype=np.float32)
    m[:FFT, :FFT] = cos.T
    m[:FFT, FFT:] = sin.T
    m[FFT:, :FFT] = -sin.T
    m[FFT:, FFT:] = cos.T

    # radix-2: even/odd sample rows; cols k<128 of both Re and Im halves
    # (cols k+128 equal these up to the sign of the odd-row block)
    rows_e = np.concatenate([np.arange(0, 256, 2), np.arange(256, 512, 2)])
    rows_o = rows_e + 1
    cols_lo = np.concatenate([np.arange(0, 128), np.arange(256, 384)])
    me = m[np.ix_(rows_e, cols_lo)]     # [256, 256]
    mg = m[np.ix_(rows_o, cols_lo)]     # [256, 256]
    m_full = np.concatenate([me, mg], axis=0)              # [512, 256] f32
    m_dev = np.ascontiguousarray(m_full[:128]).astype(BF16)
    m8_dev = np.ascontiguousarray(m_full[128:]).astype(F8)

    z = x.reshape(BATCH, C)[:, np.concatenate([rows_e, rows_o])]  # f32
    # chunk-major per-core layouts:
    # zt[cc, p, j]    = z[core*B_SHARD + cc*CB + j, p]            (bf16)
    # z8[cc, p, k, j] = z[core*B_SHARD + cc*CB + j, 128(k+1) + p] (fp8)
    z16 = z[:, :128].astype(BF16)
    zc16 = (z16.view(np.uint16)
            .reshape(N_CORES, NCH, CB, 128)
            .transpose(0, 1, 3, 2))
    zc16 = np.ascontiguousarray(zc16)   # [N_CORES, NCH, 128, CB] u16
    z8 = z[:, 128:].astype(F8)
    zc8 = (z8.view(np.uint8)
           .reshape(N_CORES, NCH, CB, 3, 128)
           .transpose(0, 1, 4, 3, 2))
    zc8 = np.ascontiguousarray(zc8)     # [N_CORES, NCH, 128, 3, CB] u8

    in_maps = []
    for c in range(N_CORES):
        in_maps.append({"zt": zc16[c].view(BF16), "z8": zc8[c].view(F8),
                        "m": m_dev, "m8": m8_dev})
    return in_maps


def _run(in_maps, trace=False):
    nc = _get_nc()
    return run_bass_kernel_spmd(nc, in_maps, list(range(N_CORES)), trace=trace)


def kernel(x, cos_kernel, sin_kernel):
    in_maps = _prepare_in_maps(x, cos_kernel, sin_kernel)
    res = _run(in_maps)
    outs = []
    for r in res.results:
        # [G, p, a, q, b] bf16 (transposed): row = G*GROUP_B + b,
        # col = q*256 + a*128 + p
        o = np.asarray(r["out"]).view(np.uint16)
        o = o.transpose(0, 4, 3, 2, 1).reshape(B_SHARD, J)  # (G,b,q,a,p)
        outs.append(o)
    out = np.concatenate(outs, axis=0).view(BF16).astype(np.float32)
    return out.reshape(BATCH, J, 1)
